# revision 7
# baseline (speedup 1.0000x reference)
"""Trainium2 Bass kernel for nn_AttnDecoder (B=8, L=128, H=512, V=32000), 8 cores.

Sharding: phase 1 (embedding/attention/GRU prep) and the GRU recurrence are
computed redundantly on all 8 cores (the GRU is stream-bound on the PE
regardless of batch count, so batch-sharding it buys nothing); phase 2 (the
[B,L,V] generation softmax + copy one-hot) is vocab-sharded 8 ways with a
single [B*L] AllReduce for the softmax denominator.

Device layouts use a transposed "feature-major" convention so the GRU gate
math runs on [128, 32] tiles and matmul stationary operands never need
on-device transposes of big tensors:
  sequence index n = t*8 + b  (t = decoder step, b = batch)
  outputT[p, hc, t, b] = gru_output[b, t, hc*128+p]
"""

import numpy as np

import concourse.bass as bass
import concourse.mybir as mybir
import concourse.tile as tile
from contextlib import ExitStack
from concourse.bass_utils import run_bass_kernel_spmd
from concourse.masks import make_identity
from concourse.vector_clock import ScopedClock
from concourse.tile import add_dep_helper

try:
    import ml_dtypes

    BF16 = ml_dtypes.bfloat16
except ImportError:  # pragma: no cover
    import jax.numpy as jnp

    BF16 = jnp.bfloat16

F32 = np.float32

B, L, H, V = 8, 128, 512, 32000
NCORES = 8
VS = V // NCORES  # vocab shard per core
VB = VS // 8      # 500: one PSUM-bank worth of vocab columns
HC = H // 128     # 4 h-chunks
NM = (B * L) // 128  # 8 row-chunks of the flattened sequence axis
AF = mybir.ActivationFunctionType
ALU = mybir.AluOpType
dt = mybir.dt

# ---------------------------------------------------------------------------
# Workarounds: this walrus build accepts at most ONE sync wait per
# instruction, while Tile emits multi-wait instructions (incl. its final
# drain). Split extra waits onto standalone same-engine NoOps.
# ---------------------------------------------------------------------------


def _patched_drain_and_barrier(self, tick_clock, wait_clock):
    probe = self.nc.sync.nop()
    wait_clock.add_sem_waits(probe.ins, ScopedClock({None: tick_clock.global_clock}))
    ws = (probe.ins.sync_info.on_wait or []) if probe.ins.sync_info else []
    if len(ws) > 1:
        probe.ins.sync_info.on_wait = ws[:1]
        for w in ws[1:]:
            nop = self.nc.sync.nop()
            if nop.ins.sync_info is None:
                nop.ins.sync_info = mybir.SyncInfo(on_wait=[w], on_update=[])
            else:
                nop.ins.sync_info.on_wait = [w]
    self.nc.sync.drain()
    self.nc.all_engine_barrier()
    popped = self.nc._tile_sem_poison_stack.pop()
    assert popped is self._sem_poison
    self.nc.clear_and_free_semaphores(list(self.sems.allocated().values()))
    self.nc.all_engine_barrier()


def _split_sync_waits(nc):
    ctr = 0
    for fn in nc.m.functions:
        for bb in fn.blocks:
            out = []
            for inst in bb.instructions:
                si = inst.sync_info
                if si is not None and si.on_wait and len(si.on_wait) > 1:
                    ws = list(si.on_wait)
                    for w in ws[:-1]:
                        ctr += 1
                        out.append(
                            mybir.InstNoOp(
                                name=f"WSPLIT-{ctr}",
                                engine=inst.engine,
                                ins=[],
                                outs=[],
                                sync_info=mybir.SyncInfo(on_wait=[w], on_update=[]),
                            )
                        )
                    si.on_wait = ws[-1:]
                    inst.sync_info = si
                out.append(inst)
            bb.instructions[:] = out
    return ctr


def _install_fixups():
    tile.TileContext._drain_and_barrier = _patched_drain_and_barrier
    try:
        from concourse import tile_utils

        if getattr(tile_utils, "max_sbuf_usage", 0) < 200 * 1024:
            tile_utils.max_sbuf_usage = 200 * 1024
    except Exception:
        pass


def _merge32(ap):
    # [128, 4, 8] (hc, b) slice with contiguous strides -> [128, 32]
    return ap.rearrange("p hc b -> p (hc b)")


def _split32(ap):
    # [128, 32] -> [128, 4, 8]
    return ap.rearrange("p (hc b) -> p hc b", hc=HC)


# ---------------------------------------------------------------------------
# Device kernel
# ---------------------------------------------------------------------------


def _build_nc(use_out_b):
    _install_fixups()
    nc = bass.Bass("TRN2", debug=False, num_devices=NCORES)

    def din(name, shape, d):
        return nc.dram_tensor(name, shape, d, kind="ExternalInput").ap()

    io = {}
    io["emb_t"] = din("emb", [V, H], dt.bfloat16)
    io["embidx"] = din("embidx", [128, NM], dt.int32)
    io["enc_t"] = din("enc", [B, L, H], dt.bfloat16)
    io["h0_t"] = din("h0", [B, H], dt.float32)
    io["awT_t"] = din("awT", [128, 8, 128], dt.bfloat16)
    io["attnb_t"] = din("attnb", [128, 1], dt.float32)
    io["cwT_t"] = din("cwT", [128, 8, H], dt.bfloat16)
    io["combb_t"] = din("combb", [128, HC], dt.float32)
    io["wihT_t"] = din("wihT", [128, HC, 3 * H], dt.bfloat16)
    io["whhT_t"] = din("whhT", [128, HC, 3 * H], dt.bfloat16)
    io["girzb_t"] = din("girzb", [128, 8], dt.float32)
    io["ginb_t"] = din("ginb", [128, HC], dt.float32)
    io["bhhn_t"] = din("bhhn", [128, HC * B], dt.float32)
    io["owT_t"] = din("owT", [128, HC, VS], dt.bfloat16)
    io["dogen_t"] = din("dogen", [128, 12], dt.bfloat16)
    io["dogenb_t"] = din("dogenb", [1, 1], dt.float32)
    io["wd_t"] = din("wd", [128, HC], dt.bfloat16)
    io["webc_t"] = din("webc", [128, H], dt.bfloat16)
    io["padc_t"] = din("padc", [128, B], dt.float32)
    io["soff_t"] = din("soff", [128, NM], dt.int32)
    io["smask_t"] = din("smask", [128, NM], dt.float32)
    io["outb_t"] = din("outb", [1, VS], dt.float32) if use_out_b else None

    io["prob_t"] = nc.dram_tensor("prob", [B, L, VS], dt.float16, kind="ExternalOutput").ap()
    io["attn_t"] = nc.dram_tensor("attn", [B, L, L], dt.float32, kind="ExternalOutput").ap()

    with tile.TileContext(nc) as tc:
        _emit(nc, tc, io)
    _split_sync_waits(nc)
    return nc


def _emit(nc, tc, io):
    bf = dt.bfloat16
    f32 = dt.float32
    f16 = dt.float16

    with ExitStack() as ctx:
        cpool = ctx.enter_context(tc.tile_pool(name="const", bufs=1))
        mpool = ctx.enter_context(tc.tile_pool(name="main", bufs=1))
        dram = ctx.enter_context(tc.tile_pool(name="dram", bufs=1, space="DRAM"))

        # ---- constants / weights resident all kernel ----
        whhT = cpool.tile([128, HC, 3 * H], bf, tag="whhT")
        owT = cpool.tile([128, HC, VS], bf, tag="owT")
        enc = cpool.tile([128, B, H], bf, tag="enc")
        dogen = cpool.tile([128, 12], bf, tag="dogen")
        wd = cpool.tile([128, HC], bf, tag="wd")
        webc = cpool.tile([128, H], bf, tag="webc")
        padc = cpool.tile([128, B], f32, tag="padc")
        attnb = cpool.tile([128, 1], f32, tag="attnb")
        combb = cpool.tile([128, HC], f32, tag="combb")
        girzb = cpool.tile([128, 8], f32, tag="girzb")
        ginb = cpool.tile([128, HC], f32, tag="ginb")
        bhhn = cpool.tile([128, HC * B], f32, tag="bhhn")
        dogenb = cpool.tile([128, 1], f32, tag="dogenb")
        idx_sb = cpool.tile([128, NM], dt.int32, tag="idx")
        soff = cpool.tile([128, NM], dt.int32, tag="soff")
        smask = cpool.tile([128, NM], f32, tag="smask")
        idf = cpool.tile([128, 128], f32, tag="idf")
        idf8 = cpool.tile([8, 8], f32, tag="idf8")
        idb = cpool.tile([128, 128], bf, tag="idb")
        ones_row = cpool.tile([1, 128], bf, tag="ones")
        h0s = cpool.tile([B, H], f32, tag="h0s")
        h0T = cpool.tile([128, HC * B], f32, tag="h0T")
        h0Tb = cpool.tile([128, HC * B], bf, tag="h0Tb")
        outb_sb = cpool.tile([1, VS], f32, tag="outb") if io["outb_t"] is not None else None

        nc.sync.dma_start(out=whhT[:], in_=io["whhT_t"][:])
        nc.sync.dma_start(out=owT[:], in_=io["owT_t"][:])
        for b in range(B):
            nc.sync.dma_start(out=enc[:, b, :], in_=io["enc_t"][b])
        nc.sync.dma_start(out=dogen[:], in_=io["dogen_t"][:])
        nc.sync.dma_start(out=wd[:], in_=io["wd_t"][:])
        nc.sync.dma_start(out=webc[:], in_=io["webc_t"][:])
        nc.sync.dma_start(out=padc[:], in_=io["padc_t"][:])
        nc.sync.dma_start(out=attnb[:], in_=io["attnb_t"][:])
        nc.sync.dma_start(out=combb[:], in_=io["combb_t"][:])
        nc.sync.dma_start(out=girzb[:], in_=io["girzb_t"][:])
        nc.sync.dma_start(out=ginb[:], in_=io["ginb_t"][:])
        nc.sync.dma_start(out=bhhn[:], in_=io["bhhn_t"][:])
        nc.sync.dma_start(out=dogenb[:], in_=io["dogenb_t"].to_broadcast((128, 1)))
        nc.sync.dma_start(out=idx_sb[:], in_=io["embidx"][:])
        nc.sync.dma_start(out=soff[:], in_=io["soff_t"][:])
        nc.sync.dma_start(out=smask[:], in_=io["smask_t"][:])
        nc.sync.dma_start(out=h0s[:], in_=io["h0_t"][:])
        if outb_sb is not None:
            nc.sync.dma_start(out=outb_sb[:], in_=io["outb_t"][:])
        make_identity(nc, idf[:])
        make_identity(nc, idf8[:])
        nc.vector.tensor_copy(out=idb[:], in_=idf[:])
        nc.vector.memset(ones_row[:], 1.0)

        # ---- tensors spanning phase 1 -> 3 ----
        embT = mpool.tile([128, HC, B * L], bf, tag="embT")
        giT = [mpool.tile([128, L, HC, B], f16, name=f"giT{g}", tag=f"giT{g}") for g in range(3)]
        outT = mpool.tile([128, HC, L, B], bf, tag="outT")
        ctxT = mpool.tile([128, HC, L, B], bf, tag="ctxT")
        erow = mpool.tile([1, B, 128], bf, tag="erow")
        zp = mpool.tile([128, NM], f32, tag="zp")
        zall = mpool.tile([128, NM], f32, tag="zall")
        mixs = mpool.tile([128, NM], f32, tag="mixs")
        s2m = mpool.tile([128, NM], f32, tag="s2m")
        lns1 = mpool.tile([128, NM], f32, tag="lns1")
        dcol = mpool.tile([128, B], f32, tag="dcol")
        za = mpool.tile([128, B], f32, tag="za")

        # ===================================================================
        # Phase 1: embedding gather, attention, xT, giT
        # ===================================================================
        with tc.tile_pool(name="p1", bufs=1) as p1, \
             tc.tile_pool(name="p1b", bufs=2) as p1b:

            awT = p1.tile([128, 8, 128], bf, tag="awT")
            cwT = p1.tile([128, 8, H], bf, tag="cwT")
            wihT = p1.tile([128, HC, 3 * H], bf, tag="wihT")
            aaT = p1.tile([128, HC, B * L], bf, tag="aaT")
            xT = p1.tile([128, HC, B * L], bf, tag="xT")
            nc.sync.dma_start(out=awT[:], in_=io["awT_t"][:])
            nc.sync.dma_start(out=cwT[:], in_=io["cwT_t"][:])
            nc.sync.dma_start(out=wihT[:], in_=io["wihT_t"][:])

            # h0T: [8,512] -> [128, (hc,b)]
            ps_tr_cm = tc.tile_pool(name="ps_tr", bufs=2, space="PSUM")
            ps_tr = ps_tr_cm.__enter__()
            for kc in range(HC):
                pt = ps_tr.tile([128, B], f32, tag="tr8")
                nc.tensor.transpose(out=pt[:], in_=h0s[:, kc * 128:(kc + 1) * 128],
                                    identity=idf8[:])
                nc.vector.tensor_copy(out=h0T[:, kc * B:(kc + 1) * B], in_=pt[:])
            nc.vector.tensor_copy(out=h0Tb[:], in_=h0T[:])

            # embedding gather + transpose into embT
            for m in range(NM):
                eg = p1b.tile([128, H], bf, tag="embg")
                nc.gpsimd.indirect_dma_start(
                    out=eg[:], out_offset=None, in_=io["emb_t"][:],
                    in_offset=bass.IndirectOffsetOnAxis(ap=idx_sb[:, m:m + 1], axis=0),
                )
                for kc in range(HC):
                    pt = ps_tr.tile([128, 128], bf, tag="tr")
                    nc.tensor.transpose(out=pt[:], in_=eg[:, kc * 128:(kc + 1) * 128],
                                        identity=idb[:])
                    nc.vector.tensor_copy(out=embT[:, kc, m * 128:(m + 1) * 128], in_=pt[:])

            ps_tr_cm.__exit__(None, None, None)

            # attention bias2[j,b] = h0 @ attn_W[:,H:].T  (+ attn_b)
            ps_att_cm = tc.tile_pool(name="ps_att", bufs=2, space="PSUM")
            ps_att = ps_att_cm.__enter__()
            pb2 = ps_att.tile([128, B], f32, tag="att3")
            for kc in range(HC):
                nc.tensor.matmul(out=pb2[:], lhsT=awT[:, 4 + kc, :],
                                 rhs=h0Tb[:, kc * B:(kc + 1) * B],
                                 start=(kc == 0), stop=(kc == HC - 1))
            b2 = p1.tile([128, B], f32, tag="b2")
            nc.vector.tensor_scalar(out=b2[:], in0=pb2[:], scalar1=attnb[:, 0:1],
                                    scalar2=None, op0=ALU.add)

            asum = p1.tile([128, B], f32, tag="asum")
            arcp = p1.tile([128, B], f32, tag="arcp")
            for b in range(B):
                embT_b = [embT[:, kc, :].rearrange("p (t b) -> p t b", b=B)[:, :, b]
                          for kc in range(HC)]
                pat = ps_att.tile([128, 128], f32, tag="att")
                for kc in range(HC):
                    nc.tensor.matmul(out=pat[:], lhsT=awT[:, kc, :], rhs=embT_b[kc],
                                     start=(kc == 0), stop=(kc == HC - 1))
                aw = p1.tile([128, 128], f32, tag="aw")
                nc.scalar.activation(out=aw[:], in_=pat[:], func=AF.Exp,
                                     bias=b2[:, b:b + 1], accum_out=asum[:, b:b + 1])
                nc.vector.reciprocal(out=arcp[:, b:b + 1], in_=asum[:, b:b + 1])
                awn = p1.tile([128, 128], f32, tag="awn")
                nc.vector.tensor_scalar(out=awn[:], in0=aw[:], scalar1=arcp[:, b:b + 1],
                                        scalar2=None, op0=ALU.mult)
                awnb = p1.tile([128, 128], bf, tag="awnb")
                nc.vector.tensor_copy(out=awnb[:], in_=awn[:])
                # attn_weights output: transpose [j,i] -> [i,j]
                pto = ps_att.tile([128, 128], f32, tag="att2")
                nc.tensor.transpose(out=pto[:], in_=awn[:], identity=idf[:])
                ao = p1b.tile([128, 128], f32, tag="ao")
                nc.vector.tensor_copy(out=ao[:], in_=pto[:])
                nc.sync.dma_start(out=io["attn_t"][b], in_=ao[:])
                # attn_appliedT
                for hc in range(HC):
                    paa = ps_att.tile([128, 128], f32, tag="att2")
                    nc.tensor.matmul(out=paa[:], lhsT=enc[:, b, hc * 128:(hc + 1) * 128],
                                     rhs=awnb[:], start=True, stop=True)
                    aaT_dst = aaT[:, hc, :].rearrange("p (t b) -> p t b", b=B)[:, :, b]
                    nc.vector.tensor_copy(out=aaT_dst, in_=paa[:])
                # e_part rows (+pad+copy_b)
                mw = p1.tile([128, H], f32, tag="mw")
                nc.vector.tensor_tensor(out=mw[:], in0=enc[:, b, :], in1=webc[:], op=ALU.mult)
                ep = p1.tile([128, 1], f32, tag="ep")
                nc.vector.tensor_reduce(out=ep[:], in_=mw[:], axis=mybir.AxisListType.X,
                                        op=ALU.add)
                epe = p1.tile([128, 1], f32, tag="epe")
                nc.vector.tensor_tensor(out=epe[:], in0=ep[:], in1=padc[:, b:b + 1], op=ALU.add)
                pte = ps_att.tile([1, 128], f32, tag="att2")
                nc.tensor.transpose(out=pte[:], in_=epe[:], identity=idf[:])
                nc.vector.tensor_copy(out=erow[0:1, b, :], in_=pte[:])

            ps_att_cm.__exit__(None, None, None)
            ps_xg_cm = tc.tile_pool(name="ps_xg", bufs=2, space="PSUM")
            ps_xg = ps_xg_cm.__enter__()

            # xT = relu(comb_W @ catT + comb_b), cat = [embT; aaT]
            for hc in range(HC):
                for hf in range(2):
                    px = ps_xg.tile([128, 512], f32, tag="px")
                    for kc2 in range(8):
                        src = embT if kc2 < 4 else aaT
                        nc.tensor.matmul(out=px[:],
                                         lhsT=cwT[:, kc2, hc * 128:(hc + 1) * 128],
                                         rhs=src[:, kc2 % 4, hf * 512:(hf + 1) * 512],
                                         start=(kc2 == 0), stop=(kc2 == 7))
                    nc.vector.tensor_scalar(out=xT[:, hc, hf * 512:(hf + 1) * 512], in0=px[:],
                                            scalar1=combb[:, hc:hc + 1], scalar2=0.0,
                                            op0=ALU.add, op1=ALU.max)

            # giT = Wih @ xT + bias  -> [128, t, hc, b] per gate
            for jc in range(12):
                g, hcg = jc // 4, jc % 4
                bias = girzb[:, jc:jc + 1] if g < 2 else ginb[:, hcg:hcg + 1]
                for hf in range(2):
                    pg = ps_xg.tile([128, 512], f32, tag="pg")
                    for kc in range(HC):
                        nc.tensor.matmul(out=pg[:],
                                         lhsT=wihT[:, kc, jc * 128:(jc + 1) * 128],
                                         rhs=xT[:, kc, hf * 512:(hf + 1) * 512],
                                         start=(kc == 0), stop=(kc == HC - 1))
                    # psum cols are n=(t,b), t in [64*hf, 64*hf+64); dest (t, hc, b)
                    nc.vector.tensor_scalar(
                        out=giT[g][:, hf * 64:(hf + 1) * 64, hcg, :],
                        in0=pg[:].rearrange("p (t b) -> p t b", b=B),
                        scalar1=bias, scalar2=None, op0=ALU.add)
            ps_xg_cm.__exit__(None, None, None)

        # ===================================================================
        # Phase 2: GRU recurrence + interleaved vocab-shard logits
        # ===================================================================
        with tc.tile_pool(name="p2", bufs=1) as p2, \
             tc.tile_pool(name="pstep", bufs=2) as pstep:

            logit = [p2.tile([128, VS], f16, name=f"logit{m}", tag=f"logit{m}") for m in range(NM)]
            prob_dma_insts = [None] * NM

            with tc.tile_pool(name="psv", bufs=2, space="PSUM") as psv, \
                 tc.tile_pool(name="ps2", bufs=2, space="PSUM") as ps2:
                hprev_f = h0T
                for t in range(L):
                    pgh = ps2.tile([128, 96], f32, tag="gh")
                    for jc in range(12):
                        for kc in range(HC):
                            rhs = (h0Tb[:, kc * B:(kc + 1) * B] if t == 0
                                   else outT[:, kc, t - 1, :])
                            nc.tensor.matmul(out=pgh[:, jc * 8:(jc + 1) * 8],
                                             lhsT=whhT[:, kc, jc * 128:(jc + 1) * 128],
                                             rhs=rhs, start=(kc == 0), stop=(kc == HC - 1))
                    rpre = pstep.tile([128, 32], f32, tag="rpre")
                    nc.vector.tensor_tensor(out=rpre[:], in0=_merge32(giT[0][:, t, :, :]),
                                            in1=pgh[:, 0:32], op=ALU.add)
                    r_t = pstep.tile([128, 32], f32, tag="r")
                    nc.scalar.activation(out=r_t[:], in_=rpre[:], func=AF.Sigmoid)
                    zpre = pstep.tile([128, 32], f32, tag="zpre")
                    nc.vector.tensor_tensor(out=zpre[:], in0=_merge32(giT[1][:, t, :, :]),
                                            in1=pgh[:, 32:64], op=ALU.add)
                    z_t = pstep.tile([128, 32], f32, tag="z")
                    nc.scalar.activation(out=z_t[:], in_=zpre[:], func=AF.Sigmoid)
                    hn = pstep.tile([128, 32], f32, tag="hn")
                    nc.vector.tensor_tensor(out=hn[:], in0=pgh[:, 64:96], in1=bhhn[:], op=ALU.add)
                    nm_ = pstep.tile([128, 32], f32, tag="nm")
                    nc.vector.tensor_tensor(out=nm_[:], in0=r_t[:], in1=hn[:], op=ALU.mult)
                    npre = pstep.tile([128, 32], f32, tag="npre")
                    nc.vector.tensor_tensor(out=npre[:], in0=nm_[:],
                                            in1=_merge32(giT[2][:, t, :, :]), op=ALU.add)
                    ns = pstep.tile([128, 32], f32, tag="ns")
                    nc.scalar.activation(out=ns[:], in_=npre[:], func=AF.Sigmoid, scale=2.0)
                    n_t = pstep.tile([128, 32], f32, tag="n")
                    nc.vector.tensor_scalar(out=n_t[:], in0=ns[:], scalar1=2.0, scalar2=-1.0,
                                            op0=ALU.mult, op1=ALU.add)
                    hd = pstep.tile([128, 32], f32, tag="hd")
                    nc.vector.tensor_tensor(out=hd[:], in0=hprev_f[:], in1=n_t[:],
                                            op=ALU.subtract)
                    zh = pstep.tile([128, 32], f32, tag="zh")
                    nc.vector.tensor_tensor(out=zh[:], in0=z_t[:], in1=hd[:], op=ALU.mult)
                    hnew = pstep.tile([128, 32], f32, tag="hcur")
                    nc.vector.tensor_tensor(out=hnew[:], in0=n_t[:], in1=zh[:], op=ALU.add)
                    nc.vector.tensor_copy(out=outT[:, :, t, :], in_=_split32(hnew[:]))
                    hprev_f = hnew

                    if t % 16 == 15:
                        m = t // 16
                        lhs_m = [outT[:, kc, m * 16:(m + 1) * 16, :]
                                 .rearrange("p t b -> p (t b)") for kc in range(HC)]
                        for vp in range(4):
                            pv = [psv.tile([128, VB], f32, name=f"pv{q}", tag=f"pv{q}") for q in range(2)]
                            for kc in range(HC):
                                first = (kc == 0)
                                last = (kc == HC - 1) and io["outb_t"] is None
                                for q in range(2):
                                    vq = vp * 2 + q
                                    nc.tensor.matmul(
                                        out=pv[q][:], lhsT=lhs_m[kc],
                                        rhs=owT[:, kc, vq * VB:(vq + 1) * VB],
                                        start=first, stop=last)
                            if outb_sb is not None:
                                for q in range(2):
                                    vq = vp * 2 + q
                                    nc.tensor.matmul(
                                        out=pv[q][:], lhsT=ones_row[:],
                                        rhs=outb_sb[:, vq * VB:(vq + 1) * VB],
                                        start=False, stop=True)
                            for q in range(2):
                                vq = vp * 2 + q
                                nc.vector.tensor_copy(
                                    out=logit[m][:, vq * VB:(vq + 1) * VB], in_=pv[q][:])

            # ===============================================================
            # Phase 3: alphas/context/mix; vocab softmax; outputs
            # ===============================================================
            with tc.tile_pool(name="ps3", bufs=1, space="PSUM") as ps3:
                # d_part columns
                for b in range(B):
                    pd = ps3.tile([128, 1], f32, tag="pd")
                    for kc in range(HC):
                        nc.tensor.matmul(out=pd[:], lhsT=outT[:, kc, :, b],
                                         rhs=wd[:, kc:kc + 1],
                                         start=(kc == 0), stop=(kc == HC - 1))
                    nc.vector.tensor_copy(out=dcol[:, b:b + 1], in_=pd[:])

                for b in range(B):
                    pal = ps3.tile([128, 128], f32, tag="pal")
                    nc.tensor.matmul(out=pal[:], lhsT=ones_row[:], rhs=erow[0:1, b, :],
                                     start=True, stop=True)
                    alf = pstep.tile([128, 128], f32, tag="alf")
                    nc.scalar.activation(out=alf[:], in_=pal[:], func=AF.Exp,
                                         bias=dcol[:, b:b + 1], accum_out=za[:, b:b + 1])
                    arz = pstep.tile([128, 1], f32, tag="arz")
                    nc.vector.reciprocal(out=arz[:], in_=za[:, b:b + 1])
                    alnb = pstep.tile([128, 128], bf, tag="alnb")
                    nc.vector.tensor_scalar(out=alnb[:], in0=alf[:], scalar1=arz[:, 0:1],
                                            scalar2=None, op0=ALU.mult)
                    ptl = ps3.tile([128, 128], bf, tag="ptl")
                    nc.tensor.transpose(out=ptl[:], in_=alnb[:], identity=idb[:])
                    alT = pstep.tile([128, 128], bf, tag="alT")
                    nc.vector.tensor_copy(out=alT[:], in_=ptl[:])
                    for hc in range(HC):
                        pc = ps3.tile([128, 128], f32, tag="pc")
                        nc.tensor.matmul(out=pc[:], lhsT=enc[:, b, hc * 128:(hc + 1) * 128],
                                         rhs=alT[:], start=True, stop=True)
                        nc.vector.tensor_copy(out=ctxT[:, hc, :, b], in_=pc[:])

                # vocab softmax pass 1: Z partials (exp values discarded)
                for m in range(NM):
                    scratch = pstep.tile([128, VS], f16, tag="pr")
                    nc.scalar.activation(out=scratch[:], in_=logit[m][:], func=AF.Exp,
                                         accum_out=zp[:, m:m + 1])

                # AllReduce Z across the 8 vocab shards
                zin_d = dram.tile([128, NM], f32, tag="zin")
                zout_d = dram.tile([128, NM], f32, tag="zout")
                nc.sync.dma_start(out=zin_d[:], in_=zp[:])
                nc.gpsimd.collective_compute(
                    "AllReduce", ALU.add, replica_groups=[list(range(NCORES))],
                    ins=[zin_d.opt()], outs=[zout_d.opt()])
                nc.sync.dma_start(out=zall[:], in_=zout_d[:])

                # mix = sigmoid(dogen @ [out; ctx; emb] + dogen_b)
                for m in range(NM):
                    pm = ps3.tile([128, 1], f32, tag="pm")
                    for c in range(12):
                        if c < 8:
                            src = (outT, ctxT)[c // 4]
                            lhsT = src[:, c % 4, m * 16:(m + 1) * 16, :] \
                                .rearrange("p t b -> p (t b)")
                        else:
                            lhsT = embT[:, c % 4, m * 128:(m + 1) * 128]
                        nc.tensor.matmul(out=pm[:], lhsT=lhsT, rhs=dogen[:, c:c + 1],
                                         start=(c == 0), stop=(c == 11))
                    nc.scalar.activation(out=mixs[:, m:m + 1], in_=pm[:], func=AF.Sigmoid,
                                         bias=dogenb[:, 0:1])

                # s2m = (1 - mix) * in_shard_mask ; lns1 = ln(mix) - ln(Z)
                s2 = p2.tile([128, NM], f32, tag="s2")
                nc.vector.tensor_scalar(out=s2[:], in0=mixs[:], scalar1=-1.0, scalar2=1.0,
                                        op0=ALU.mult, op1=ALU.add)
                nc.vector.tensor_tensor(out=s2m[:], in0=s2[:], in1=smask[:], op=ALU.mult)
                s2mh = p2.tile([128, NM], f16, tag="s2mh")
                nc.vector.tensor_copy(out=s2mh[:], in_=s2m[:])
                lnm = p2.tile([128, NM], f32, tag="lnm")
                lnz = p2.tile([128, NM], f32, tag="lnz")
                nc.scalar.activation(out=lnm[:], in_=mixs[:], func=AF.Ln)
                nc.scalar.activation(out=lnz[:], in_=zall[:], func=AF.Ln)
                nc.vector.tensor_tensor(out=lns1[:], in0=lnm[:], in1=lnz[:], op=ALU.subtract)

                # pass 2: prob = exp(logit + ln(mix/Z)) -> DMA out
                for m in range(NM):
                    pr = pstep.tile([128, VS], f16, tag="pr")
                    nc.scalar.activation(out=pr[:], in_=logit[m][:], func=AF.Exp,
                                         bias=lns1[:, m:m + 1])
                    t0 = m * 16
                    di = nc.sync.dma_start(
                        out=io["prob_t"][:, t0:t0 + 16, :].rearrange("b t v -> t b v"),
                        in_=pr[:])
                    prob_dma_insts[m] = di.ins

                # one-hot copy-dist scatter: prob[flat_off] += s2m (element RMW)
                pflat = io["prob_t"].rearrange("b t v -> (b t v)").unsqueeze(1)
                for m in range(NM):
                    g = pstep.tile([128, 1], f16, tag="g")
                    gi_ = nc.gpsimd.indirect_dma_start(
                        out=g[:], out_offset=None, in_=pflat,
                        in_offset=bass.IndirectOffsetOnAxis(ap=soff[:, m:m + 1], axis=0))
                    add_dep_helper(gi_.ins, prob_dma_insts[m], sync=True,
                                   reason="rmw after prob write")
                    g2 = pstep.tile([128, 1], f16, tag="g2")
                    nc.vector.tensor_tensor(out=g2[:], in0=g[:], in1=s2mh[:, m:m + 1],
                                            op=ALU.add)
                    nc.gpsimd.indirect_dma_start(
                        out=pflat,
                        out_offset=bass.IndirectOffsetOnAxis(ap=soff[:, m:m + 1], axis=0),
                        in_=g2[:], in_offset=None)


# ---------------------------------------------------------------------------
# Host side
# ---------------------------------------------------------------------------


def _prepare_inputs(inputs):
    inp = {k: np.asarray(v) for k, v in inputs.items()}
    input_ = inp["input_"].astype(np.int64)
    enc_in = inp["encoder_inputs"].astype(np.int64)
    h0 = inp["encoder_hidden"][0].astype(F32)            # [B,H]
    enc = inp["encoder_outputs"].astype(BF16)            # [B,L,H]
    emb = inp["emb_table"].astype(BF16)                  # [V,H]
    attn_W = inp["attn_W"].astype(F32)                   # [L, 2H]
    attn_b = inp["attn_b"].astype(F32)
    comb_W = inp["comb_W"].astype(F32)                   # [H, 2H]
    comb_b = inp["comb_b"].astype(F32)
    Wih = inp["gru_Wih"].astype(F32)                     # [3H, H]
    Whh = inp["gru_Whh"].astype(F32)
    bih = inp["gru_bih"].astype(F32)
    bhh = inp["gru_bhh"].astype(F32)
    out_W = inp["out_W"].astype(F32)                     # [V, H]
    out_b = inp["out_b"].astype(F32)
    dogen_W = inp["dogen_W"].astype(F32)                 # [1, 3H]
    dogen_b = inp["dogen_b"].astype(F32)
    copy_W = inp["copy_W"].astype(F32)                   # [1, 2H]
    copy_b = inp["copy_b"].astype(F32)

    def swz(mat_T, nchunk):
        # [K, J] -> [128, K//128, J] with p = k % 128 (k = kc*128 + p)
        K, J = mat_T.shape
        return np.ascontiguousarray(mat_T.reshape(nchunk, 128, J).transpose(1, 0, 2))

    awT = swz(attn_W.T.astype(BF16), 8)                  # [128, 8, 128]
    cwT = swz(comb_W.T.astype(BF16), 8)                  # [128, 8, 512]
    wihT = swz(Wih.T.astype(BF16), HC)                   # [128, 4, 1536]
    whhT = swz(Whh.T.astype(BF16), HC)                   # [128, 4, 1536]

    grz = (bih + bhh)[: 2 * H]
    girzb = np.ascontiguousarray(grz.reshape(8, 128).T).astype(F32)          # [128, 8]
    ginb = np.ascontiguousarray(bih[2 * H:].reshape(HC, 128).T).astype(F32)  # [128, 4]
    bhhn = np.repeat(bhh[2 * H:].reshape(HC, 128).T[:, :, None], B, axis=2)
    bhhn = np.ascontiguousarray(bhhn.reshape(128, HC * B)).astype(F32)       # [128, 32]

    combb = np.ascontiguousarray(comb_b.reshape(HC, 128).T).astype(F32)      # [128, 4]
    attnb = attn_b.reshape(128, 1).astype(F32)

    dg = dogen_W[0]
    dogen = np.stack([dg[c * 128:(c + 1) * 128] for c in range(12)], axis=1).astype(BF16)
    wd = np.stack([copy_W[0, H + c * 128: H + (c + 1) * 128] for c in range(HC)],
                  axis=1).astype(BF16)
    webc = np.tile(copy_W[0, :H][None, :], (128, 1)).astype(BF16)            # [128, 512]
    padc = (np.where(enc_in == 0, -1000.0, 0.0).T + copy_b[0]).astype(F32)   # [128(j), 8]

    # gather indices, n = t*8 + b; chunk m covers t in [16m, 16m+16)
    n_idx = input_.T  # [t, b]
    embidx = np.ascontiguousarray(n_idx.reshape(NM, 16 * B).T).astype(np.int32)  # [128, NM]

    use_out_b = bool(np.any(out_b != 0.0))

    base = dict(
        emb=emb, embidx=embidx,
        enc=np.ascontiguousarray(enc), h0=h0,
        awT=awT, attnb=attnb, cwT=cwT, combb=combb,
        wihT=wihT, whhT=whhT, girzb=girzb, ginb=ginb, bhhn=bhhn,
        dogen=dogen, dogenb=dogen_b.reshape(1, 1).astype(F32),
        wd=wd, webc=webc, padc=np.ascontiguousarray(padc),
    )

    t_g, b_g = np.meshgrid(np.arange(L), np.arange(B), indexing="ij")  # [t, b]
    tgt_tb = enc_in.T  # [t, b] vocab ids
    in_maps = []
    for c in range(NCORES):
        v0 = c * VS
        owT_c = swz(out_W[v0:v0 + VS].T.astype(BF16), HC)  # [128, 4, 4000]
        vv = tgt_tb - v0
        ok = (vv >= 0) & (vv < VS)
        flat = (b_g * L + t_g) * VS + np.where(ok, vv, 0)
        soff_c = np.ascontiguousarray(flat.reshape(NM, 128).T).astype(np.int32)
        smask_c = np.ascontiguousarray(ok.astype(np.float32).reshape(NM, 128).T)
        m = dict(base)
        m["owT"] = owT_c
        m["soff"] = soff_c
        m["smask"] = smask_c
        if use_out_b:
            m["outb"] = out_b[v0:v0 + VS].reshape(1, VS).astype(F32)
        in_maps.append(m)
    return in_maps, use_out_b


_NC_CACHE = {}


def kernel(**inputs):
    in_maps, use_out_b = _prepare_inputs(inputs)
    if use_out_b not in _NC_CACHE:
        _NC_CACHE[use_out_b] = _build_nc(use_out_b)
    nc = _NC_CACHE[use_out_b]
    res = run_bass_kernel_spmd(nc, in_maps, list(range(NCORES)))
    prob = np.concatenate(
        [res.results[c]["prob"].astype(np.float32) for c in range(NCORES)], axis=2)
    attn = res.results[0]["attn"].astype(np.float32)
    return prob, attn
